# revision 77
# baseline (speedup 1.0000x reference)
"""Mixtral GQA attention block (B=1, S=2048, HID=4096, NH=32, NKV=8, HD=128),
8-way tensor-parallel over heads on trn2: each core owns 4 query heads + 1 KV
head (one GQA group), computes its partial output projection, host sums the
8 partials.

Device layout notes (v2):
  - All matmul operands staged transposed (contraction dim on partitions);
    host pre-packs partition-major so DMAs are identity copies.
  - Scores computed transposed (S^T[k,q]) so exp'd tiles serve as lhsT for
    P@V; softmax denominator rides along as a ones-column on V.
  - RoPE runs entirely on DVE: the half-swap is expressed as two
    partition-shifted multiplies (output base partition differs from the
    inputs', which the ISA allows) against a sign-folded sin table.
  - V and attention-output transposes use the DMA xbar (dma_start_transpose),
    keeping the PE free for real matmuls.
  - The emission order software-pipelines the whole kernel: output-projection
    matmuls of chunk I-1 (and first-chunk projections of chunk I+1) are
    interleaved as fillers into the attention phase of chunk I so the PE
    never waits on the score->exp->PV dependency chain; exp is issued in two
    256-wide halves so the first PV can start earlier.
"""

import math
import os
import sys
from collections import deque

import numpy as np

sys.path.insert(0, "/opt/trn_rl_repo")

import concourse.bass as bass
import concourse.tile as tile
from concourse import bacc
from concourse import mybir

S = 2048
HID = 4096
NH, NKV, HD = 32, 8, 128
NCORES = 8
QH = NH // NCORES      # 4 query heads per core
ND = HID // 128        # 32 contraction chunks
NI = S // 512          # 4 q-chunks of 512
NJ = S // 128          # 16 k-tiles of 128
SCALE = 1.0 / math.sqrt(HD)

F16 = mybir.dt.float16
BF16 = mybir.dt.bfloat16
F32 = mybir.dt.float32

# PE rows of work to cover the score->exp->PV chain latency per iteration
# (per chunk; later chunks have more iterations than filler supply, so
# spread the fillers thinner there)
CHAIN_TAB = [4000, 1500, 1500, 1700]

_CACHE = {}
LAST_RESULTS = None

DEBUG_LABELS = {}
MM_LABELS = []
_CUR_LABEL = [""]


def _build_program():
    nc = bacc.Bacc()

    # record a label for every emitted instruction (any engine) so the
    # trace analyzer can attribute stalls; no effect on the program
    for eng in (nc.gpsimd, nc.scalar, nc.tensor, nc.vector, nc.sync):
        try:
            orig = eng.add_instruction

            def _wrap(inst, _orig=orig):
                r = _orig(inst)
                try:
                    DEBUG_LABELS[inst.name] = _CUR_LABEL[0]
                except Exception:
                    pass
                return r

            eng.add_instruction = _wrap
        except Exception:
            pass

    def mm(*a, **k):
        nc.tensor.matmul(*a, **k)
        MM_LABELS.append(_CUR_LABEL[0])

    ht = nc.declare_dram_parameter("ht", [128, ND, S], F16, isOutput=False)
    wq = nc.declare_dram_parameter("wq", [128, QH, ND, 128], F16,
                                   isOutput=False)
    wk = nc.declare_dram_parameter("wk", [128, ND, 128], F16, isOutput=False)
    wv = nc.declare_dram_parameter("wv", [128, ND, 128], F16, isOutput=False)
    wo = nc.declare_dram_parameter("wo", [128, QH, HID], F16, isOutput=False)
    cosd = nc.declare_dram_parameter("cosd", [128, S], F16, isOutput=False)
    sinsw = nc.declare_dram_parameter("sinsw", [128, S], F16, isOutput=False)
    maskd = nc.declare_dram_parameter("maskd", [128, 128], F16, isOutput=False)
    identd = nc.declare_dram_parameter("identd", [128, 128], F16, isOutput=False)
    out = nc.declare_dram_parameter("out", [S, HID], F16, isOutput=True)

    with tile.TileContext(nc) as tc:
        with (
            tc.tile_pool(name="consts", bufs=1) as consts,
            tc.tile_pool(name="hpool", bufs=12) as hpool,
            tc.tile_pool(name="ptpool", bufs=7) as ptpool,
            tc.tile_pool(name="rtmp", bufs=2) as rtmp,
            tc.tile_pool(name="small", bufs=8) as small,
            tc.tile_pool(name="orow", bufs=8) as orowp,
            tc.tile_pool(name="pproj", bufs=4, space="PSUM") as pproj,
            tc.tile_pool(name="pwork", bufs=2, space="PSUM") as pwork,
            tc.tile_pool(name="ppv", bufs=2, space="PSUM") as ppv,
        ):
            wq_sb = consts.tile([128, QH, ND, 128], F16)
            cos_sb = consts.tile([128, S], F16)
            sin_sb = consts.tile([128, S], F16)
            wk_sb = consts.tile([128, ND, 128], F16)
            wv_sb = consts.tile([128, ND, 128], F16)
            wo_sb = consts.tile([128, QH, HID], F16)
            maskneg = consts.tile([128, 128], F16)
            ident16 = consts.tile([128, 128], F16)

            # per-chunk tiles so cross-chunk pipelining cannot create
            # false dependencies between writers and readers
            qT = [consts.tile([128, QH, 512], F16, name=f"qT{i}")
                  for i in range(NI)]
            kT = [consts.tile([128, 512], F16, name=f"kT{i}")
                  for i in range(NI)]
            # V' tiles: per k-tile j, [128 tokens, 128 ch + ones column].
            # One tile per j: dma_start_transpose requires offset-0
            # whole-tile targets on both sides.
            vA = [consts.tile([128, 132], BF16, name=f"vA{j}")
                  for j in range(NJ)]
            for j in range(NJ):
                nc.vector.memset(vA[j][:, 128:129], 1.0)
            attnT = [[[consts.tile([128, 128], F16, name=f"aT{i}_{t}_{il}")
                       for il in range(4)] for t in range(QH)]
                     for i in range(NI)]

            # ---- ht tiles: ring of 12 [128, 4, 512] tiles; chunk I uses
            # slots (8I..8I+7) mod 12, prefetched well ahead of use.
            ht_tiles = {}

            def ht_dma(I, dq, split=False, eng=None):
                _CUR_LABEL[0] = f"dma.ht{I}.{dq}"
                t_h = hpool.tile([128, 4, 512], F16, tag="ht",
                                 name=f"ht_{I}_{dq}")
                ht_tiles[(I, dq)] = t_h
                src = ht[:, dq * 4:(dq + 1) * 4, I * 512:(I + 1) * 512]
                eng = eng or nc.sync
                if split:
                    eng.dma_start(out=t_h[:, 0:2, :], in_=src[:, 0:2, :])
                    eng.dma_start(out=t_h[:, 2:4, :], in_=src[:, 2:4, :])
                else:
                    eng.dma_start(out=t_h, in_=src)

            # ---- initial DMA issue -----------------------------------
            # sync queue: weights, striped so the startup-critical pieces
            # land first.  gpsimd (SWDGE) queue: ht tiles.  scalar queue:
            # rope tables.
            def wq_dma(tt, dh):
                nc.sync.dma_start(
                    out=wq_sb[:, tt, dh * 16:(dh + 1) * 16, :],
                    in_=wq[:, tt, dh * 16:(dh + 1) * 16, :],
                )

            def wk_dma(s8):
                nc.sync.dma_start(out=wk_sb[:, s8 * 8:(s8 + 1) * 8, :],
                                  in_=wk[:, s8 * 8:(s8 + 1) * 8, :])

            def wv_dma(s8):
                nc.sync.dma_start(out=wv_sb[:, s8 * 8:(s8 + 1) * 8, :],
                                  in_=wv[:, s8 * 8:(s8 + 1) * 8, :])

            # one strictly-ordered stream on the sync queue, interleaved in
            # the exact order the chunk-0 staggered wavefront consumes it
            wk_dma(0)
            nc.sync.dma_start(out=maskneg, in_=maskd[:, :])
            nc.sync.dma_start(out=ident16, in_=identd[:, :])
            ht_dma(0, 0, split=True)
            wv_dma(0)
            ht_dma(0, 1)
            wq_dma(0, 0)
            nc.scalar.dma_start(out=cos_sb, in_=cosd[:, :])
            nc.scalar.dma_start(out=sin_sb, in_=sinsw[:, :])
            ht_dma(0, 2)
            wk_dma(1)
            wv_dma(1)
            wq_dma(1, 0)
            ht_dma(0, 3)
            wk_dma(2)
            wv_dma(2)
            ht_dma(0, 4)
            wq_dma(2, 0)
            wk_dma(3)
            wv_dma(3)
            ht_dma(0, 5)
            wq_dma(0, 1)
            ht_dma(0, 6)
            wq_dma(1, 1)
            ht_dma(0, 7)
            wq_dma(2, 1)
            wq_dma(3, 0)
            wq_dma(3, 1)
            # ht(1) prefetch ahead of wo: wo is not needed until the
            # first output projection (during B(1)).
            for dq in range(8):
                ht_dma(1, dq)
            def wo_dma(i8):
                oc, dh = divmod(i8, 2)
                nc.sync.dma_start(
                    out=wo_sb[:, oc, dh * 2048:(dh + 1) * 2048],
                    in_=wo[:, oc, dh * 2048:(dh + 1) * 2048],
                )

            # ---- helpers ---------------------------------------------
            def rope_into(ps, dst, nsl, w=512, lbl=""):
                """ps: PSUM [128, w] f32 pre-rope (transposed layout).
                dst: f16 SBUF slice.  DVE-only: the half-swap is two
                partition-shifted multiplies against the sign-folded sin
                table (out base partition differs from the inputs')."""
                _CUR_LABEL[0] = f"rope.{lbl}"
                cpy = rtmp.tile([128, 512], F16, tag="ropecpy")
                nc.scalar.copy(cpy[:, :w], ps[:, :w])
                t1 = rtmp.tile([128, 512], F16, tag="ropet1")
                nc.vector.tensor_mul(t1[:, :w], cpy[:, :w], cos_sb[:, nsl])
                t2 = rtmp.tile([128, 512], F16, tag="ropet2")
                nc.vector.tensor_mul(
                    t2[0:64, :w], cpy[64:128, :w], sin_sb[64:128, nsl]
                )
                nc.vector.tensor_mul(
                    t2[64:128, :w], cpy[0:64, :w], sin_sb[0:64, nsl]
                )
                nc.vector.tensor_add(dst, t1[:, :w], t2[:, :w])

            def finish_k(ps, I):
                rope_into(ps, kT[I][:, :],
                          slice(I * 512, (I + 1) * 512), lbl=f"k{I}")

            def finish_q(ps, I, t):
                rope_into(ps, qT[I][:, t, :],
                          slice(I * 512, (I + 1) * 512), lbl=f"q{I}.{t}")

            def finish_v(ps, I):
                _CUR_LABEL[0] = f"vtr.{I}"
                for jj in range(4):
                    vt = small.tile([128, 128], BF16, tag="vt", bufs=4)
                    nc.vector.tensor_copy(vt, ps[:, jj * 128:(jj + 1) * 128])
                    nc.scalar.dma_start_transpose(vA[4 * I + jj][:, 0:128],
                                                  vt)

            def proj_units(I):
                """A(I): list of (emitfn, rows) PE-matmul units in emission
                order.  D-major passes over output groups; rope /
                v-transpose work is attached to each group's closing unit.
                Chunk 0 uses one wide 5-group pass (borrowing the idle
                attention-phase PSUM banks) so the per-ht-tile consumption
                rate stays below the streaming DMA rate."""
                units = []
                state = {}

                def w_slice(g, d):
                    if g == "k":
                        return wk_sb[:, d, :]
                    if g == "v":
                        return wv_sb[:, d, :]
                    t = int(g[1])
                    return wq_sb[:, t, d, :]

                def alloc(g):
                    if I == 0 and g == "q2":
                        return ppv.tile([128, 512], F32, tag="pv",
                                        name=f"proj_{I}_{g}")
                    if I == 0 and g in ("q1", "q3"):
                        return pwork.tile([128, 512], F32, tag="work",
                                          name=f"proj_{I}_{g}")
                    return pproj.tile([128, 512], F32, tag="proj",
                                      name=f"proj_{I}_{g}")

                def finish(g, ps):
                    if g == "k":
                        finish_k(ps, I)
                    elif g == "v":
                        finish_v(ps, I)
                    else:
                        finish_q(ps, I, int(g[1]))

                def mk(g, d):
                    def emit():
                        if d == 0:
                            state[g] = alloc(g)
                        ps = state[g]
                        _CUR_LABEL[0] = f"A{I}.{g}.d{d}"
                        mm(
                            ps, w_slice(g, d),
                            ht_tiles[(I, d // 4)][:, d % 4, :],
                            start=(d == 0), stop=(d == ND - 1),
                        )
                        if d == ND - 1:
                            finish(g, ps)
                        return 512
                    return emit

                if I == 0:
                    # staggered wavefront: group g starts one ht-tile later
                    # than the previous, smoothing the early DMA demand
                    offs = {"k": 0, "v": 0, "q0": 1, "q1": 2, "q2": 3}
                    for w in range(8 + 3 + 1):
                        for g, og in offs.items():
                            if 0 <= w - og < 8:
                                for d in range(4 * (w - og), 4 * (w - og) + 4):
                                    units.append(("A", mk(g, d), 512))
                    for d in range(ND):
                        units.append(("A", mk("q3", d), 512))
                else:
                    for pas in (("k", "v"), ("q0", "q1"), ("q2", "q3")):
                        for d in range(ND):
                            for g in pas:
                                units.append(("A", mk(g, d), 512))
                return units

            def c_units(I):
                """C(I): output-projection units (one PE matmul each)."""
                units = []
                state = {}

                def mk(il, qtr, mc, oc):
                    i_abs = 4 * I + il
                    mq = qtr * 1024 + mc * 512

                    def emit():
                        if oc == 0:
                            state[(il, qtr, mc)] = pproj.tile(
                                [128, 512], F32, tag="proj",
                                name=f"oproj_{I}_{il}_{qtr}_{mc}")
                        ps = state[(il, qtr, mc)]
                        _CUR_LABEL[0] = f"C{I}.il{il}.q{qtr}.m{mc}.oc{oc}"
                        mm(
                            ps,
                            attnT[I][oc][il],
                            wo_sb[:, oc, mq:mq + 512],
                            start=(oc == 0), stop=(oc == QH - 1),
                        )
                        if oc == QH - 1:
                            # stage via the idle GPSIMD engine (so the copy
                            # cannot block DVE's finalize/mask work), and
                            # defer its emission by one pop so its wait on
                            # the closing matmul is already satisfied
                            ps_done = ps

                            def flush(ps=ps_done, i_abs=i_abs, mq=mq,
                                      lbl=f"Cflush{I}.il{il}.q{qtr}.m{mc}"):
                                _CUR_LABEL[0] = lbl
                                orow = orowp.tile([128, 512], F16,
                                                  tag="orow")
                                nc.vector.tensor_copy(orow, ps)
                                nc.sync.dma_start(
                                    out=out[i_abs * 128:(i_abs + 1) * 128,
                                            mq:mq + 512],
                                    in_=orow,
                                )
                            deferred.append((0, flush))
                        return 512
                    return emit

                groups = [(il, qtr, mc)
                          for il in range(4)
                          for qtr in range(4)
                          for mc in range(2)]
                for il, qtr, mc in groups:
                    for oc in range(QH):
                        units.append(("C", mk(il, qtr, mc, oc), 512))
                return units

            deferred = deque()

            def pop_fillers(fillers, need_rows):
                # age-based deferral: a flush only emits after 2 pop rounds,
                # so the next PSUM-ring alloc's snapshot-wait lands on a
                # flush whose closing matmul has long executed
                for _ in range(len(deferred)):
                    age, fn = deferred.popleft()
                    if age >= 1:
                        fn()
                    else:
                        deferred.append((age + 1, fn))
                while need_rows > 0 and fillers:
                    _, fn, rows = fillers.popleft()
                    fn()
                    need_rows -= rows

            def emit_B(I, fillers):
                njt = 4 * I + 4
                state = {}

                def finalize_pair(t, h):
                    # copy the closed pv pair to SBUF in one shot (so no DVE
                    # read ever touches the PSUM tile mid-accumulation),
                    # then normalize both halves from SBUF
                    _CUR_LABEL[0] = f"B{I}.fin.t{t}.h{h}"
                    pv = state[t]["pv"][h]
                    fsb = small.tile([128, 2, 129], F32, tag="finsb",
                                     bufs=3)
                    nc.vector.tensor_copy(fsb, pv)
                    for hh in range(2):
                        il = 2 * h + hh
                        recip = small.tile([128, 1], F32, tag="recip",
                                           bufs=4)
                        nc.vector.reciprocal(recip, fsb[:, hh, 128:129])
                        osc = small.tile([128, 128], F16, tag="osc", bufs=6)
                        nc.vector.tensor_scalar_mul(
                            osc, fsb[:, hh, 0:128], recip
                        )
                        state[t]["oscs"][il] = osc

                def drain(t, j, pt, fillers=None):
                    o_pv = state[t]["pv"]
                    for il in range(4):
                        i_abs = 4 * I + il
                        if j <= i_abs:
                            _CUR_LABEL[0] = f"B{I}.pv.t{t}.j{j}.il{il}"
                            # start=True resets the ENTIRE psum bank, so
                            # only the even sibling of each pair issues it
                            # (wiping the bank for both); the odd sibling
                            # accumulates onto the zeroed region
                            mm(
                                o_pv[il // 2][:, il % 2, 0:129],
                                pt[:, il * 128:(il + 1) * 128],
                                vA[j][:, 0:129],
                                start=(j == 0 and il % 2 == 0),
                                stop=(j == i_abs),
                                skip_group_check=True,
                            )
                            if j == i_abs and il % 2 == 1:
                                finalize_pair(t, il // 2)

                def head_epilogue(t):
                    _CUR_LABEL[0] = f"B{I}.tp.t{t}"
                    last = I == NI - 1 and t == QH - 1
                    for il in range(4):
                        if last:
                            # final head: PE transpose + DVE copy slots in
                            # right behind the last PV with ~600ns latency,
                            # vs ~5us through the DMA queues -- the tail's
                            # oc3 matmuls are gated on this
                            tps = pwork.tile([128, 512], F16, tag="work",
                                             name=f"tps_{I}_{t}_{il}")
                            nc.tensor.transpose(
                                tps[:, 0:128], state[t]["oscs"][il],
                                ident16,
                            )
                            nc.vector.tensor_copy(
                                attnT[I][t][il], tps[:, 0:128]
                            )
                        else:
                            nc.sync.dma_start_transpose(
                                attnT[I][t][il], state[t]["oscs"][il]
                            )

                prev = None
                it_idx = 0
                for t in range(QH):
                    state[t] = {
                        "pv": [
                            ppv.tile([128, 2, 129], F32, tag="pv",
                                     name=f"pv_{I}_{t}_{h}")
                            for h in range(2)
                        ],
                        "oscs": [None] * 4,
                    }
                    for j in range(njt):
                        # spread next chunk's ht prefetch (and chunk 0's
                        # wo load) across the early iterations instead of a
                        # single burst
                        if (I >= 1 and I + 1 < NI and it_idx % 2 == 0
                                and 2 <= it_idx <= 16):
                            ht_dma(I + 1, it_idx // 2 - 1, eng=nc.gpsimd)
                        if I == 0 and 1 <= it_idx <= 8:
                            wo_dma(it_idx - 1)
                        it_idx += 1
                        m = j - 4 * I
                        q_off = 128 * m if m > 0 else 0
                        s_ps = pwork.tile([128, 512], F32, tag="work")
                        _CUR_LABEL[0] = f"B{I}.s.t{t}.j{j}"
                        mm(
                            s_ps[:, q_off:512],
                            kT[j // 4][:, (j % 4) * 128:(j % 4 + 1) * 128],
                            qT[I][:, t, q_off:512],
                            start=True, stop=(m < 0),
                        )
                        if m >= 0:
                            # causal bias: -30000 on the below-diagonal
                            # entries of the boundary tile; exp underflows
                            # to an exact 0, so no mask multiply is needed
                            _CUR_LABEL[0] = f"B{I}.sm.t{t}.j{j}"
                            mm(
                                s_ps[:, q_off:q_off + 128],
                                ident16,
                                maskneg,
                                start=False, stop=True,
                                skip_group_check=True,
                            )
                        boost = 1024 if (I >= 1 and t > 0 and j <= 1) else 0
                        pop_fillers(
                            fillers,
                            CHAIN_TAB[I] + boost - (512 - q_off),
                        )
                        # drain the previous tile BEFORE emitting this tile's
                        # exp/mask: keeps DVE finalizes ahead of the mask in
                        # the in-order DVE queue
                        if prev is not None:
                            tp, jp, ptp = prev
                            drain(tp, jp, ptp, fillers)
                            if jp == njt - 1:
                                head_epilogue(tp)
                        # exp in two 256-wide halves so PV(il 0/1) of this
                        # tile can start before the whole row is exp'd
                        _CUR_LABEL[0] = f"B{I}.exp.t{t}.j{j}"
                        pt = ptpool.tile([128, 512], BF16, tag="pt")
                        if q_off < 384:
                            nc.scalar.activation(
                                pt[:, q_off:384], s_ps[:, q_off:384],
                                mybir.ActivationFunctionType.Exp,
                                scale=SCALE,
                            )
                        nc.scalar.activation(
                            pt[:, max(384, q_off):512],
                            s_ps[:, max(384, q_off):512],
                            mybir.ActivationFunctionType.Exp,
                            scale=SCALE,
                        )
                        prev = (t, j, pt)
                tp, jp, ptp = prev
                drain(tp, jp, ptp)
                head_epilogue(tp)

            # ---- top-level schedule ----------------------------------
            fillers = deque()
            for _, fn, rows in proj_units(0):
                fn()
            for I in range(NI):
                if I + 1 < NI:
                    fillers.extend(proj_units(I + 1))
                emit_B(I, fillers)
                while deferred:
                    deferred.popleft()[1]()
                # A(I+1) remainder must be fully emitted before B(I+1);
                # C leftovers stay queued for the next attention phase.
                rem = list(fillers)
                fillers.clear()
                for kind, fn, rows in rem:
                    if kind == "A":
                        fn()
                        while deferred:
                            deferred.popleft()[1]()
                    else:
                        fillers.append((kind, fn, rows))
                fillers.extend(c_units(I))
            tail_i = 0
            for _, fn, rows in fillers:
                for _ in range(len(deferred)):
                    age, dfn = deferred.popleft()
                    if age >= 1:
                        dfn()
                    else:
                        deferred.append((age + 1, dfn))
                fn()
                tail_i += 1
            while deferred:
                deferred.popleft()[1]()
    nc.finalize()
    return nc


def _pack_inputs(h, position_ids, wq, wk, wv, wo):
    """Host-side shard + transpose + cast. Returns per-core input maps."""
    import ml_dtypes

    ht = np.ascontiguousarray(
        h.T.reshape(ND, 128, S).transpose(1, 0, 2)
    ).astype(np.float16)

    # RoPE tables in transposed orientation; sin sign-folded for the
    # partition-shifted DVE half-swap.
    inv = 1.0 / (1e6 ** (np.arange(0, HD, 2, dtype=np.float64) / HD))
    fr = position_ids.astype(np.float64)[None, :] * inv[:, None]   # [64, S]
    cosT = np.cos(fr).astype(np.float16)
    sinT = np.sin(fr).astype(np.float16)
    cosd = np.concatenate([cosT, cosT], axis=0)                    # [128, S]
    sinsw = np.concatenate([sinT, -sinT], axis=0)
    p_i = np.arange(128)[:, None]
    f_i = np.arange(128)[None, :]
    maskd = np.where(f_i - p_i >= 0, 0.0, -30000.0).astype(np.float16)
    identd = np.eye(128, dtype=np.float16)

    in_maps = []
    for c in range(NCORES):
        wq_c = wq[c * 512:(c + 1) * 512, :]          # [512, HID]
        wk_c = wk[c * 128:(c + 1) * 128, :]
        wv_c = wv[c * 128:(c + 1) * 128, :]
        wo_c = wo[:, c * 512:(c + 1) * 512]          # [HID, 512]
        in_maps.append({
            "ht": ht,
            "wq": np.ascontiguousarray(
                wq_c.T.reshape(ND, 128, QH, 128).transpose(1, 2, 0, 3)
            ).astype(np.float16),
            "wk": np.ascontiguousarray(
                wk_c.T.reshape(ND, 128, 128).transpose(1, 0, 2)
            ).astype(np.float16),
            "wv": np.ascontiguousarray(
                wv_c.T.reshape(ND, 128, 128).transpose(1, 0, 2)
            ).astype(np.float16),
            "wo": np.ascontiguousarray(
                wo_c.T.reshape(QH, 128, HID).transpose(1, 0, 2)
            ).astype(np.float16),
            "cosd": cosd,
            "sinsw": sinsw,
            "maskd": maskd,
            "identd": identd,
        })
    return in_maps


def kernel(h, position_ids, wq, wk, wv, wo):
    global LAST_RESULTS
    from concourse.bass_utils import run_bass_kernel_spmd

    if "nc" not in _CACHE:
        _CACHE["nc"] = _build_program()
    nc = _CACHE["nc"]

    in_maps = _pack_inputs(
        np.asarray(h, dtype=np.float32),
        np.asarray(position_ids),
        np.asarray(wq, dtype=np.float32),
        np.asarray(wk, dtype=np.float32),
        np.asarray(wv, dtype=np.float32),
        np.asarray(wo, dtype=np.float32),
    )

    trace = bool(int(os.environ.get("KERNEL_TRACE", "0")))
    res = run_bass_kernel_spmd(
        nc, in_maps, core_ids=list(range(NCORES)), trace=trace
    )
    LAST_RESULTS = res

    acc = np.zeros((S, HID), dtype=np.float32)
    for r in res.results:
        acc += r["out"].astype(np.float32)
    return acc


# revision 78
# speedup vs baseline: 1.0035x; 1.0035x over previous
"""Mixtral GQA attention block (B=1, S=2048, HID=4096, NH=32, NKV=8, HD=128),
8-way tensor-parallel over heads on trn2: each core owns 4 query heads + 1 KV
head (one GQA group), computes its partial output projection, host sums the
8 partials.

Device layout notes (v2):
  - All matmul operands staged transposed (contraction dim on partitions);
    host pre-packs partition-major so DMAs are identity copies.
  - Scores computed transposed (S^T[k,q]) so exp'd tiles serve as lhsT for
    P@V; softmax denominator rides along as a ones-column on V.
  - RoPE runs entirely on DVE: the half-swap is expressed as two
    partition-shifted multiplies (output base partition differs from the
    inputs', which the ISA allows) against a sign-folded sin table.
  - V and attention-output transposes use the DMA xbar (dma_start_transpose),
    keeping the PE free for real matmuls.
  - The emission order software-pipelines the whole kernel: output-projection
    matmuls of chunk I-1 (and first-chunk projections of chunk I+1) are
    interleaved as fillers into the attention phase of chunk I so the PE
    never waits on the score->exp->PV dependency chain; exp is issued in two
    256-wide halves so the first PV can start earlier.
"""

import math
import os
import sys
from collections import deque

import numpy as np

sys.path.insert(0, "/opt/trn_rl_repo")

import concourse.bass as bass
import concourse.tile as tile
from concourse import bacc
from concourse import mybir

S = 2048
HID = 4096
NH, NKV, HD = 32, 8, 128
NCORES = 8
QH = NH // NCORES      # 4 query heads per core
ND = HID // 128        # 32 contraction chunks
NI = S // 512          # 4 q-chunks of 512
NJ = S // 128          # 16 k-tiles of 128
SCALE = 1.0 / math.sqrt(HD)

F16 = mybir.dt.float16
BF16 = mybir.dt.bfloat16
F32 = mybir.dt.float32

# PE rows of work to cover the score->exp->PV chain latency per iteration
# (per chunk; later chunks have more iterations than filler supply, so
# spread the fillers thinner there)
CHAIN_TAB = [4000, 1400, 1400, 1600]

_CACHE = {}
LAST_RESULTS = None

DEBUG_LABELS = {}
MM_LABELS = []
_CUR_LABEL = [""]


def _build_program():
    nc = bacc.Bacc()

    # record a label for every emitted instruction (any engine) so the
    # trace analyzer can attribute stalls; no effect on the program
    for eng in (nc.gpsimd, nc.scalar, nc.tensor, nc.vector, nc.sync):
        try:
            orig = eng.add_instruction

            def _wrap(inst, _orig=orig):
                r = _orig(inst)
                try:
                    DEBUG_LABELS[inst.name] = _CUR_LABEL[0]
                except Exception:
                    pass
                return r

            eng.add_instruction = _wrap
        except Exception:
            pass

    def mm(*a, **k):
        nc.tensor.matmul(*a, **k)
        MM_LABELS.append(_CUR_LABEL[0])

    ht = nc.declare_dram_parameter("ht", [128, ND, S], F16, isOutput=False)
    wq = nc.declare_dram_parameter("wq", [128, QH, ND, 128], F16,
                                   isOutput=False)
    wk = nc.declare_dram_parameter("wk", [128, ND, 128], F16, isOutput=False)
    wv = nc.declare_dram_parameter("wv", [128, ND, 128], F16, isOutput=False)
    wo = nc.declare_dram_parameter("wo", [128, QH, HID], F16, isOutput=False)
    cosd = nc.declare_dram_parameter("cosd", [128, S], F16, isOutput=False)
    sinsw = nc.declare_dram_parameter("sinsw", [128, S], F16, isOutput=False)
    maskd = nc.declare_dram_parameter("maskd", [128, 128], F16, isOutput=False)
    identd = nc.declare_dram_parameter("identd", [128, 128], F16, isOutput=False)
    out = nc.declare_dram_parameter("out", [S, HID], F16, isOutput=True)

    with tile.TileContext(nc) as tc:
        with (
            tc.tile_pool(name="consts", bufs=1) as consts,
            tc.tile_pool(name="hpool", bufs=12) as hpool,
            tc.tile_pool(name="ptpool", bufs=7) as ptpool,
            tc.tile_pool(name="rtmp", bufs=2) as rtmp,
            tc.tile_pool(name="small", bufs=8) as small,
            tc.tile_pool(name="orow", bufs=8) as orowp,
            tc.tile_pool(name="pproj", bufs=4, space="PSUM") as pproj,
            tc.tile_pool(name="pwork", bufs=2, space="PSUM") as pwork,
            tc.tile_pool(name="ppv", bufs=2, space="PSUM") as ppv,
        ):
            wq_sb = consts.tile([128, QH, ND, 128], F16)
            cos_sb = consts.tile([128, S], F16)
            sin_sb = consts.tile([128, S], F16)
            wk_sb = consts.tile([128, ND, 128], F16)
            wv_sb = consts.tile([128, ND, 128], F16)
            wo_sb = consts.tile([128, QH, HID], F16)
            maskneg = consts.tile([128, 128], F16)
            ident16 = consts.tile([128, 128], F16)

            # per-chunk tiles so cross-chunk pipelining cannot create
            # false dependencies between writers and readers
            qT = [consts.tile([128, QH, 512], F16, name=f"qT{i}")
                  for i in range(NI)]
            kT = [consts.tile([128, 512], F16, name=f"kT{i}")
                  for i in range(NI)]
            # V' tiles: per k-tile j, [128 tokens, 128 ch + ones column].
            # One tile per j: dma_start_transpose requires offset-0
            # whole-tile targets on both sides.
            vA = [consts.tile([128, 132], BF16, name=f"vA{j}")
                  for j in range(NJ)]
            for j in range(NJ):
                nc.vector.memset(vA[j][:, 128:129], 1.0)
            attnT = [[[consts.tile([128, 128], F16, name=f"aT{i}_{t}_{il}")
                       for il in range(4)] for t in range(QH)]
                     for i in range(NI)]

            # ---- ht tiles: ring of 12 [128, 4, 512] tiles; chunk I uses
            # slots (8I..8I+7) mod 12, prefetched well ahead of use.
            ht_tiles = {}

            def ht_dma(I, dq, split=False, eng=None):
                _CUR_LABEL[0] = f"dma.ht{I}.{dq}"
                t_h = hpool.tile([128, 4, 512], F16, tag="ht",
                                 name=f"ht_{I}_{dq}")
                ht_tiles[(I, dq)] = t_h
                src = ht[:, dq * 4:(dq + 1) * 4, I * 512:(I + 1) * 512]
                eng = eng or nc.sync
                if split:
                    eng.dma_start(out=t_h[:, 0:2, :], in_=src[:, 0:2, :])
                    eng.dma_start(out=t_h[:, 2:4, :], in_=src[:, 2:4, :])
                else:
                    eng.dma_start(out=t_h, in_=src)

            # ---- initial DMA issue -----------------------------------
            # sync queue: weights, striped so the startup-critical pieces
            # land first.  gpsimd (SWDGE) queue: ht tiles.  scalar queue:
            # rope tables.
            def wq_dma(tt, dh):
                nc.sync.dma_start(
                    out=wq_sb[:, tt, dh * 16:(dh + 1) * 16, :],
                    in_=wq[:, tt, dh * 16:(dh + 1) * 16, :],
                )

            def wk_dma(s8):
                nc.sync.dma_start(out=wk_sb[:, s8 * 8:(s8 + 1) * 8, :],
                                  in_=wk[:, s8 * 8:(s8 + 1) * 8, :])

            def wv_dma(s8):
                nc.sync.dma_start(out=wv_sb[:, s8 * 8:(s8 + 1) * 8, :],
                                  in_=wv[:, s8 * 8:(s8 + 1) * 8, :])

            # one strictly-ordered stream on the sync queue, interleaved in
            # the exact order the chunk-0 staggered wavefront consumes it
            wk_dma(0)
            nc.sync.dma_start(out=maskneg, in_=maskd[:, :])
            nc.sync.dma_start(out=ident16, in_=identd[:, :])
            ht_dma(0, 0, split=True)
            wv_dma(0)
            ht_dma(0, 1)
            wq_dma(0, 0)
            nc.scalar.dma_start(out=cos_sb, in_=cosd[:, :])
            nc.scalar.dma_start(out=sin_sb, in_=sinsw[:, :])
            ht_dma(0, 2)
            wk_dma(1)
            wv_dma(1)
            wq_dma(1, 0)
            ht_dma(0, 3)
            wk_dma(2)
            wv_dma(2)
            ht_dma(0, 4)
            wq_dma(2, 0)
            wk_dma(3)
            wv_dma(3)
            ht_dma(0, 5)
            wq_dma(0, 1)
            ht_dma(0, 6)
            wq_dma(1, 1)
            ht_dma(0, 7)
            wq_dma(2, 1)
            wq_dma(3, 0)
            wq_dma(3, 1)
            # ht(1) prefetch ahead of wo: wo is not needed until the
            # first output projection (during B(1)).
            for dq in range(8):
                ht_dma(1, dq)
            def wo_dma(i8):
                oc, dh = divmod(i8, 2)
                nc.sync.dma_start(
                    out=wo_sb[:, oc, dh * 2048:(dh + 1) * 2048],
                    in_=wo[:, oc, dh * 2048:(dh + 1) * 2048],
                )

            # ---- helpers ---------------------------------------------
            def rope_into(ps, dst, nsl, w=512, lbl=""):
                """ps: PSUM [128, w] f32 pre-rope (transposed layout).
                dst: f16 SBUF slice.  DVE-only: the half-swap is two
                partition-shifted multiplies against the sign-folded sin
                table (out base partition differs from the inputs')."""
                _CUR_LABEL[0] = f"rope.{lbl}"
                cpy = rtmp.tile([128, 512], F16, tag="ropecpy")
                nc.scalar.copy(cpy[:, :w], ps[:, :w])
                t1 = rtmp.tile([128, 512], F16, tag="ropet1")
                nc.vector.tensor_mul(t1[:, :w], cpy[:, :w], cos_sb[:, nsl])
                t2 = rtmp.tile([128, 512], F16, tag="ropet2")
                nc.vector.tensor_mul(
                    t2[0:64, :w], cpy[64:128, :w], sin_sb[64:128, nsl]
                )
                nc.vector.tensor_mul(
                    t2[64:128, :w], cpy[0:64, :w], sin_sb[0:64, nsl]
                )
                nc.vector.tensor_add(dst, t1[:, :w], t2[:, :w])

            def finish_k(ps, I):
                rope_into(ps, kT[I][:, :],
                          slice(I * 512, (I + 1) * 512), lbl=f"k{I}")

            def finish_q(ps, I, t):
                rope_into(ps, qT[I][:, t, :],
                          slice(I * 512, (I + 1) * 512), lbl=f"q{I}.{t}")

            def finish_v(ps, I):
                _CUR_LABEL[0] = f"vtr.{I}"
                for jj in range(4):
                    vt = small.tile([128, 128], BF16, tag="vt", bufs=4)
                    nc.vector.tensor_copy(vt, ps[:, jj * 128:(jj + 1) * 128])
                    nc.scalar.dma_start_transpose(vA[4 * I + jj][:, 0:128],
                                                  vt)

            def proj_units(I):
                """A(I): list of (emitfn, rows) PE-matmul units in emission
                order.  D-major passes over output groups; rope /
                v-transpose work is attached to each group's closing unit.
                Chunk 0 uses one wide 5-group pass (borrowing the idle
                attention-phase PSUM banks) so the per-ht-tile consumption
                rate stays below the streaming DMA rate."""
                units = []
                state = {}

                def w_slice(g, d):
                    if g == "k":
                        return wk_sb[:, d, :]
                    if g == "v":
                        return wv_sb[:, d, :]
                    t = int(g[1])
                    return wq_sb[:, t, d, :]

                def alloc(g):
                    if I == 0 and g == "q2":
                        return ppv.tile([128, 512], F32, tag="pv",
                                        name=f"proj_{I}_{g}")
                    if I == 0 and g in ("q1", "q3"):
                        return pwork.tile([128, 512], F32, tag="work",
                                          name=f"proj_{I}_{g}")
                    return pproj.tile([128, 512], F32, tag="proj",
                                      name=f"proj_{I}_{g}")

                def finish(g, ps):
                    if g == "k":
                        finish_k(ps, I)
                    elif g == "v":
                        finish_v(ps, I)
                    else:
                        finish_q(ps, I, int(g[1]))

                def mk(g, d):
                    def emit():
                        if d == 0:
                            state[g] = alloc(g)
                        ps = state[g]
                        _CUR_LABEL[0] = f"A{I}.{g}.d{d}"
                        mm(
                            ps, w_slice(g, d),
                            ht_tiles[(I, d // 4)][:, d % 4, :],
                            start=(d == 0), stop=(d == ND - 1),
                        )
                        if d == ND - 1:
                            finish(g, ps)
                        return 512
                    return emit

                if I == 0:
                    # staggered wavefront: group g starts one ht-tile later
                    # than the previous, smoothing the early DMA demand
                    offs = {"k": 0, "v": 0, "q0": 1, "q1": 2, "q2": 3}
                    for w in range(8 + 3 + 1):
                        for g, og in offs.items():
                            if 0 <= w - og < 8:
                                for d in range(4 * (w - og), 4 * (w - og) + 4):
                                    units.append(("A", mk(g, d), 512))
                    for d in range(ND):
                        units.append(("A", mk("q3", d), 512))
                else:
                    for pas in (("k", "v"), ("q0", "q1"), ("q2", "q3")):
                        for d in range(ND):
                            for g in pas:
                                units.append(("A", mk(g, d), 512))
                return units

            def c_units(I):
                """C(I): output-projection units (one PE matmul each)."""
                units = []
                state = {}

                def mk(il, qtr, mc, oc):
                    i_abs = 4 * I + il
                    mq = qtr * 1024 + mc * 512

                    def emit():
                        if oc == 0:
                            state[(il, qtr, mc)] = pproj.tile(
                                [128, 512], F32, tag="proj",
                                name=f"oproj_{I}_{il}_{qtr}_{mc}")
                        ps = state[(il, qtr, mc)]
                        _CUR_LABEL[0] = f"C{I}.il{il}.q{qtr}.m{mc}.oc{oc}"
                        mm(
                            ps,
                            attnT[I][oc][il],
                            wo_sb[:, oc, mq:mq + 512],
                            start=(oc == 0), stop=(oc == QH - 1),
                        )
                        if oc == QH - 1:
                            # stage via the idle GPSIMD engine (so the copy
                            # cannot block DVE's finalize/mask work), and
                            # defer its emission by one pop so its wait on
                            # the closing matmul is already satisfied
                            ps_done = ps

                            def flush(ps=ps_done, i_abs=i_abs, mq=mq,
                                      lbl=f"Cflush{I}.il{il}.q{qtr}.m{mc}"):
                                _CUR_LABEL[0] = lbl
                                orow = orowp.tile([128, 512], F16,
                                                  tag="orow")
                                nc.vector.tensor_copy(orow, ps)
                                nc.sync.dma_start(
                                    out=out[i_abs * 128:(i_abs + 1) * 128,
                                            mq:mq + 512],
                                    in_=orow,
                                )
                            deferred.append((0, flush))
                        return 512
                    return emit

                groups = [(il, qtr, mc)
                          for il in range(4)
                          for qtr in range(4)
                          for mc in range(2)]
                for il, qtr, mc in groups:
                    for oc in range(QH):
                        units.append(("C", mk(il, qtr, mc, oc), 512))
                return units

            deferred = deque()

            def pop_fillers(fillers, need_rows):
                # age-based deferral: a flush only emits after 2 pop rounds,
                # so the next PSUM-ring alloc's snapshot-wait lands on a
                # flush whose closing matmul has long executed
                for _ in range(len(deferred)):
                    age, fn = deferred.popleft()
                    if age >= 1:
                        fn()
                    else:
                        deferred.append((age + 1, fn))
                while need_rows > 0 and fillers:
                    _, fn, rows = fillers.popleft()
                    fn()
                    need_rows -= rows

            def emit_B(I, fillers):
                njt = 4 * I + 4
                state = {}

                def finalize_pair(t, h):
                    # copy the closed pv pair to SBUF in one shot (so no DVE
                    # read ever touches the PSUM tile mid-accumulation),
                    # then normalize both halves from SBUF
                    _CUR_LABEL[0] = f"B{I}.fin.t{t}.h{h}"
                    pv = state[t]["pv"][h]
                    fsb = small.tile([128, 2, 129], F32, tag="finsb",
                                     bufs=3)
                    nc.vector.tensor_copy(fsb, pv)
                    for hh in range(2):
                        il = 2 * h + hh
                        recip = small.tile([128, 1], F32, tag="recip",
                                           bufs=4)
                        nc.vector.reciprocal(recip, fsb[:, hh, 128:129])
                        osc = small.tile([128, 128], F16, tag="osc", bufs=6)
                        nc.vector.tensor_scalar_mul(
                            osc, fsb[:, hh, 0:128], recip
                        )
                        state[t]["oscs"][il] = osc

                def drain(t, j, pt, fillers=None):
                    o_pv = state[t]["pv"]
                    for il in range(4):
                        i_abs = 4 * I + il
                        if j <= i_abs:
                            _CUR_LABEL[0] = f"B{I}.pv.t{t}.j{j}.il{il}"
                            # start=True resets the ENTIRE psum bank, so
                            # only the even sibling of each pair issues it
                            # (wiping the bank for both); the odd sibling
                            # accumulates onto the zeroed region
                            mm(
                                o_pv[il // 2][:, il % 2, 0:129],
                                pt[:, il * 128:(il + 1) * 128],
                                vA[j][:, 0:129],
                                start=(j == 0 and il % 2 == 0),
                                stop=(j == i_abs),
                                skip_group_check=True,
                            )
                            if j == i_abs and il % 2 == 1:
                                finalize_pair(t, il // 2)

                def head_epilogue(t):
                    _CUR_LABEL[0] = f"B{I}.tp.t{t}"
                    last = I == NI - 1 and t == QH - 1
                    for il in range(4):
                        if last:
                            # final head: PE transpose + DVE copy slots in
                            # right behind the last PV with ~600ns latency,
                            # vs ~5us through the DMA queues -- the tail's
                            # oc3 matmuls are gated on this
                            tps = pwork.tile([128, 512], F16, tag="work",
                                             name=f"tps_{I}_{t}_{il}")
                            nc.tensor.transpose(
                                tps[:, 0:128], state[t]["oscs"][il],
                                ident16,
                            )
                            nc.vector.tensor_copy(
                                attnT[I][t][il], tps[:, 0:128]
                            )
                        else:
                            nc.sync.dma_start_transpose(
                                attnT[I][t][il], state[t]["oscs"][il]
                            )

                prev = None
                it_idx = 0
                for t in range(QH):
                    state[t] = {
                        "pv": [
                            ppv.tile([128, 2, 129], F32, tag="pv",
                                     name=f"pv_{I}_{t}_{h}")
                            for h in range(2)
                        ],
                        "oscs": [None] * 4,
                    }
                    for j in range(njt):
                        # spread next chunk's ht prefetch (and chunk 0's
                        # wo load) across the early iterations instead of a
                        # single burst
                        if (I >= 1 and I + 1 < NI and it_idx % 2 == 0
                                and 2 <= it_idx <= 16):
                            ht_dma(I + 1, it_idx // 2 - 1, eng=nc.gpsimd)
                        if I == 0 and 1 <= it_idx <= 8:
                            wo_dma(it_idx - 1)
                        it_idx += 1
                        m = j - 4 * I
                        q_off = 128 * m if m > 0 else 0
                        s_ps = pwork.tile([128, 512], F32, tag="work")
                        _CUR_LABEL[0] = f"B{I}.s.t{t}.j{j}"
                        mm(
                            s_ps[:, q_off:512],
                            kT[j // 4][:, (j % 4) * 128:(j % 4 + 1) * 128],
                            qT[I][:, t, q_off:512],
                            start=True, stop=(m < 0),
                        )
                        if m >= 0:
                            # causal bias: -30000 on the below-diagonal
                            # entries of the boundary tile; exp underflows
                            # to an exact 0, so no mask multiply is needed
                            _CUR_LABEL[0] = f"B{I}.sm.t{t}.j{j}"
                            mm(
                                s_ps[:, q_off:q_off + 128],
                                ident16,
                                maskneg,
                                start=False, stop=True,
                                skip_group_check=True,
                            )
                        boost = 1024 if (I >= 1 and t > 0 and j <= 1) else 0
                        pop_fillers(
                            fillers,
                            CHAIN_TAB[I] + boost - (512 - q_off),
                        )
                        # drain the previous tile BEFORE emitting this tile's
                        # exp/mask: keeps DVE finalizes ahead of the mask in
                        # the in-order DVE queue
                        if prev is not None:
                            tp, jp, ptp = prev
                            drain(tp, jp, ptp, fillers)
                            if jp == njt - 1:
                                head_epilogue(tp)
                        # exp in two 256-wide halves so PV(il 0/1) of this
                        # tile can start before the whole row is exp'd
                        _CUR_LABEL[0] = f"B{I}.exp.t{t}.j{j}"
                        pt = ptpool.tile([128, 512], BF16, tag="pt")
                        if q_off < 384:
                            nc.scalar.activation(
                                pt[:, q_off:384], s_ps[:, q_off:384],
                                mybir.ActivationFunctionType.Exp,
                                scale=SCALE,
                            )
                        nc.scalar.activation(
                            pt[:, max(384, q_off):512],
                            s_ps[:, max(384, q_off):512],
                            mybir.ActivationFunctionType.Exp,
                            scale=SCALE,
                        )
                        prev = (t, j, pt)
                tp, jp, ptp = prev
                drain(tp, jp, ptp)
                head_epilogue(tp)

            # ---- top-level schedule ----------------------------------
            fillers = deque()
            for _, fn, rows in proj_units(0):
                fn()
            for I in range(NI):
                if I + 1 < NI:
                    fillers.extend(proj_units(I + 1))
                emit_B(I, fillers)
                while deferred:
                    deferred.popleft()[1]()
                # A(I+1) remainder must be fully emitted before B(I+1);
                # C leftovers stay queued for the next attention phase.
                rem = list(fillers)
                fillers.clear()
                for kind, fn, rows in rem:
                    if kind == "A":
                        fn()
                        while deferred:
                            deferred.popleft()[1]()
                    else:
                        fillers.append((kind, fn, rows))
                fillers.extend(c_units(I))
            tail_i = 0
            for _, fn, rows in fillers:
                for _ in range(len(deferred)):
                    age, dfn = deferred.popleft()
                    if age >= 1:
                        dfn()
                    else:
                        deferred.append((age + 1, dfn))
                fn()
                tail_i += 1
            while deferred:
                deferred.popleft()[1]()
    nc.finalize()
    return nc


def _pack_inputs(h, position_ids, wq, wk, wv, wo):
    """Host-side shard + transpose + cast. Returns per-core input maps."""
    import ml_dtypes

    ht = np.ascontiguousarray(
        h.T.reshape(ND, 128, S).transpose(1, 0, 2)
    ).astype(np.float16)

    # RoPE tables in transposed orientation; sin sign-folded for the
    # partition-shifted DVE half-swap.
    inv = 1.0 / (1e6 ** (np.arange(0, HD, 2, dtype=np.float64) / HD))
    fr = position_ids.astype(np.float64)[None, :] * inv[:, None]   # [64, S]
    cosT = np.cos(fr).astype(np.float16)
    sinT = np.sin(fr).astype(np.float16)
    cosd = np.concatenate([cosT, cosT], axis=0)                    # [128, S]
    sinsw = np.concatenate([sinT, -sinT], axis=0)
    p_i = np.arange(128)[:, None]
    f_i = np.arange(128)[None, :]
    maskd = np.where(f_i - p_i >= 0, 0.0, -30000.0).astype(np.float16)
    identd = np.eye(128, dtype=np.float16)

    in_maps = []
    for c in range(NCORES):
        wq_c = wq[c * 512:(c + 1) * 512, :]          # [512, HID]
        wk_c = wk[c * 128:(c + 1) * 128, :]
        wv_c = wv[c * 128:(c + 1) * 128, :]
        wo_c = wo[:, c * 512:(c + 1) * 512]          # [HID, 512]
        in_maps.append({
            "ht": ht,
            "wq": np.ascontiguousarray(
                wq_c.T.reshape(ND, 128, QH, 128).transpose(1, 2, 0, 3)
            ).astype(np.float16),
            "wk": np.ascontiguousarray(
                wk_c.T.reshape(ND, 128, 128).transpose(1, 0, 2)
            ).astype(np.float16),
            "wv": np.ascontiguousarray(
                wv_c.T.reshape(ND, 128, 128).transpose(1, 0, 2)
            ).astype(np.float16),
            "wo": np.ascontiguousarray(
                wo_c.T.reshape(QH, 128, HID).transpose(1, 0, 2)
            ).astype(np.float16),
            "cosd": cosd,
            "sinsw": sinsw,
            "maskd": maskd,
            "identd": identd,
        })
    return in_maps


def kernel(h, position_ids, wq, wk, wv, wo):
    global LAST_RESULTS
    from concourse.bass_utils import run_bass_kernel_spmd

    if "nc" not in _CACHE:
        _CACHE["nc"] = _build_program()
    nc = _CACHE["nc"]

    in_maps = _pack_inputs(
        np.asarray(h, dtype=np.float32),
        np.asarray(position_ids),
        np.asarray(wq, dtype=np.float32),
        np.asarray(wk, dtype=np.float32),
        np.asarray(wv, dtype=np.float32),
        np.asarray(wo, dtype=np.float32),
    )

    trace = bool(int(os.environ.get("KERNEL_TRACE", "0")))
    res = run_bass_kernel_spmd(
        nc, in_maps, core_ids=list(range(NCORES)), trace=trace
    )
    LAST_RESULTS = res

    acc = np.zeros((S, HID), dtype=np.float32)
    for r in res.results:
        acc += r["out"].astype(np.float32)
    return acc
